# revision 4
# baseline (speedup 1.0000x reference)
"""Trainium2 Bass/Tile kernel: two chained VALID 3x3 convolutions.

    x  [N,3,256,256] --conv(w1)--> h [N,64,254,254] --conv(w2)--> out [N,128,252,252]

Data-parallel over 8 NeuronCores: batch N=16 -> 2 images per core, conv
weights replicated.  Per core the convs are computed as implicit GEMMs on the
tensor engine.  The kernel is tensor-engine issue-rate bound (the HW activity
monitor clamps the PE to ~1.2 GHz column rate under sustained load), so the
design minimizes total matmul *columns*:

  conv1: contraction over C0*3*3=27 on SBUF partitions (im2col buffer built
         with 9 strided DMAs), one matmul per 2-row output chunk
         (1 column per output pixel -- already minimal).
  conv2: contraction over C1*9=576 done in 5 matmul passes per 2-row chunk
         (vs 9 naive, 6 with one shifted copy):
           H  = [A;  B ]: A = h rows, B = h shifted down 1 row
                -> pairs (0,dj)+(1,dj), dj=0..2, K=128      (3 passes)
           H2 = [D;  A2]: D = h shifted left 1 col, A2 = h copy
                -> pair (2,1)+(2,0) at K=128                (1 pass)
                -> single (2,2) via D shifted one more col, K=64 (1 pass)
         The B/A2/D copies are SBUF->SBUF DMAs off the critical path.
         PSUM accumulates the 5 matmuls, DVE copies to SBUF (bf16),
         DMA to HBM; host casts the bf16 output back to fp32.

MODE selects the matmul dtype:
  "bf16": inputs cast to bfloat16 host-side, fp32 PSUM accumulation
  "tf32": float32r
"""

from contextlib import ExitStack

import ml_dtypes
import numpy as np

import concourse.bass as bass
import concourse.mybir as mybir
import concourse.tile as tile
import concourse.bass_utils as bass_utils
from concourse import bacc

N_CORES = 8
FULL_N = 16
C0, C1, C2 = 3, 64, 128

MODE = "bf16"


def _mm_dt():
    return mybir.dt.bfloat16 if MODE == "bf16" else mybir.dt.float32r


def _np_dt():
    return ml_dtypes.bfloat16 if MODE == "bf16" else np.float32


class Geom:
    def __init__(self, npc, h0, w0, ty):
        self.npc = npc          # images per core
        self.h0, self.w0 = h0, w0
        self.h1, self.w1 = h0 - 2, w0 - 2
        self.h2, self.w2 = h0 - 4, w0 - 4
        self.ty = ty            # conv2 output rows per strip
        assert ty % 2 == 0 and self.h2 % ty == 0


GEOM = Geom(npc=FULL_N // N_CORES, h0=256, w0=256, ty=28)


def _emit(ctx: ExitStack, tc: tile.TileContext, g: Geom, out, x, w1t, w2p, w2q,
          w2r, mm_dt):
    nc = tc.nc
    f32 = mybir.dt.float32
    TY, W1, W2 = g.ty, g.w1, g.w2

    wpool = ctx.enter_context(tc.tile_pool(name="weights", bufs=1))
    b1pool = ctx.enter_context(tc.tile_pool(name="b1", bufs=2))
    hpool = ctx.enter_context(tc.tile_pool(name="h", bufs=2))
    h2pool = ctx.enter_context(tc.tile_pool(name="h2", bufs=2))
    opool = ctx.enter_context(tc.tile_pool(name="o2", bufs=4))
    ps1 = ctx.enter_context(tc.tile_pool(name="ps1", bufs=2, space="PSUM"))
    ps2 = ctx.enter_context(tc.tile_pool(name="ps2", bufs=4, space="PSUM"))

    w1t_sb = wpool.tile([27, C1], mm_dt)
    nc.sync.dma_start(w1t_sb[:], w1t)
    w2p_sb = wpool.tile([128, 3, C2], mm_dt)
    nc.sync.dma_start(w2p_sb[:], w2p)
    w2q_sb = wpool.tile([128, C2], mm_dt)
    nc.sync.dma_start(w2q_sb[:], w2q)
    w2r_sb = wpool.tile([C1, C2], mm_dt)
    nc.sync.dma_start(w2r_sb[:], w2r)

    def conv1(n, y0):
        """Produce the H (=[A;B]) and H2 (=[D;A2]) strips for conv2 rows
        [y0, y0+TY)."""
        # im2col: partition (di*3+dj)*3+c holds x[c, y0+r+di, dj:dj+W1]
        B1 = b1pool.tile([27, TY + 2, W1], mm_dt, tag="b1")
        for t9 in range(9):
            di, dj = divmod(t9, 3)
            nc.sync.dma_start(
                B1[3 * t9:3 * t9 + 3],
                x[n, :, y0 + di:y0 + di + TY + 2, dj:dj + W1])
        H = hpool.tile([128, TY + 2, W1], mm_dt, tag="h")
        H2 = h2pool.tile([128, TY + 2, W1], mm_dt, tag="h2")
        for r in range(0, TY + 2, 2):
            P1 = ps1.tile([C1, 2, W1], f32, tag="p1")
            nc.tensor.matmul(P1[:], w1t_sb[:], B1[:, r:r + 2, :],
                             start=True, stop=True)
            # casting copy into partitions 0:64 (h rows r, r+1)
            nc.vector.tensor_copy(H[0:C1, r:r + 2, :], P1[:])
            # B: row-shifted copy (B row r' = h row r'+1); DMA cannot read
            # PSUM, so source the freshly written A rows
            if r == 0:
                nc.sync.dma_start(H[C1:128, 0:1, :], H[0:C1, 1:2, :])
            else:
                nc.sync.dma_start(H[C1:128, r - 1:r + 1, :], H[0:C1, r:r + 2, :])
            if r >= 2:
                # conv2 only reads H2 rows 2..TY+1
                # D: col-shifted copy (D col x = h col x+1)
                nc.sync.dma_start(H2[0:C1, r:r + 2, 0:W1 - 1],
                                  H[0:C1, r:r + 2, 1:W1])
                # A2: plain copy of h
                nc.sync.dma_start(H2[C1:128, r:r + 2, :], H[0:C1, r:r + 2, :])
        return H, H2

    def conv2(n, y0, H, H2):
        for t in range(0, TY, 2):
            P2 = ps2.tile([C2, 2, W2], f32, tag="p2")
            for dj in range(3):  # pairs: taps (0,dj) + (1,dj), K=128
                nc.tensor.matmul(P2[:], w2p_sb[:, dj, :],
                                 H[:, t:t + 2, dj:dj + W2],
                                 start=(dj == 0), stop=False)
            # pair: taps (2,1) [D] + (2,0) [A2], K=128
            nc.tensor.matmul(P2[:], w2q_sb[:],
                             H2[:, t + 2:t + 4, 0:W2],
                             start=False, stop=False)
            # single: tap (2,2) via D shifted one more col, K=64
            nc.tensor.matmul(P2[:], w2r_sb[:],
                             H2[0:C1, t + 2:t + 4, 1:1 + W2],
                             start=False, stop=True)
            O2 = opool.tile([C2, 2, W2], mm_dt, tag="o2")
            nc.vector.tensor_copy(O2[:], P2[:])
            nc.sync.dma_start(out[n, :, y0 + t:y0 + t + 2, :], O2[:])

    strips = [(n, y0) for n in range(g.npc) for y0 in range(0, g.h2, TY)]
    # software pipeline: emit conv1 of strip s+1 before conv2 of strip s so
    # the tensor engine never waits on the h copies of the strip it consumes
    Hcur = conv1(*strips[0])
    for i, (n, y0) in enumerate(strips):
        Hnext = conv1(*strips[i + 1]) if i + 1 < len(strips) else None
        conv2(n, y0, *Hcur)
        Hcur = Hnext


def build(g: Geom = GEOM, mm_dt=None):
    if mm_dt is None:
        mm_dt = _mm_dt()
    nc = bacc.Bacc("TRN2", target_bir_lowering=False, debug=False,
                   num_devices=N_CORES)
    x = nc.dram_tensor("x", [g.npc, C0, g.h0, g.w0], mm_dt,
                       kind="ExternalInput").ap()
    w1t = nc.dram_tensor("w1t", [27, C1], mm_dt, kind="ExternalInput").ap()
    w2p = nc.dram_tensor("w2p", [128, 3, C2], mm_dt, kind="ExternalInput").ap()
    w2q = nc.dram_tensor("w2q", [128, C2], mm_dt, kind="ExternalInput").ap()
    w2r = nc.dram_tensor("w2r", [C1, C2], mm_dt, kind="ExternalInput").ap()
    out = nc.dram_tensor("out", [g.npc, C2, g.h2, g.w2], mm_dt,
                         kind="ExternalOutput").ap()
    with tile.TileContext(nc) as tc:
        with ExitStack() as ctx:
            _emit(ctx, tc, g, out, x, w1t, w2p, w2q, w2r, mm_dt)
    nc.compile()
    return nc


def host_round(a: np.ndarray) -> np.ndarray:
    """Cast fp32 to the matmul storage dtype (bf16 cast, or tf32 rounding)."""
    a = np.ascontiguousarray(a, dtype=np.float32)
    if MODE == "bf16":
        return a.astype(ml_dtypes.bfloat16)
    b = a.view(np.uint32).copy()
    b += 0xFFF + ((b >> 13) & 1)
    b &= np.uint32(0xFFFFE000)
    return b.view(np.float32)


def pack_weights(w1: np.ndarray, w2: np.ndarray):
    """Host-side repack so every device DMA is contiguous.

    w1t[p, o] = w1[o, c, di, dj] with p = (di*3+dj)*3 + c  (matches im2col)
    w2p[k, dj, o]: k<64 -> w2[o, k, 0, dj];  k>=64 -> w2[o, k-64, 1, dj]
    w2q[k, o]:     k<64 -> w2[o, k, 2, 1];   k>=64 -> w2[o, k-64, 2, 0]
    w2r[c, o] = w2[o, c, 2, 2]
    """
    w1 = np.ascontiguousarray(np.asarray(w1), dtype=np.float32)
    w2 = np.ascontiguousarray(np.asarray(w2), dtype=np.float32)
    w1t = np.ascontiguousarray(w1.transpose(2, 3, 1, 0).reshape(27, C1))
    w2p = np.empty((128, 3, C2), np.float32)
    w2p[:C1] = w2[:, :, 0, :].transpose(1, 2, 0)
    w2p[C1:] = w2[:, :, 1, :].transpose(1, 2, 0)
    w2q = np.empty((128, C2), np.float32)
    w2q[:C1] = w2[:, :, 2, 1].transpose(1, 0)
    w2q[C1:] = w2[:, :, 2, 0].transpose(1, 0)
    w2r = np.ascontiguousarray(w2[:, :, 2, 2].transpose(1, 0))
    return (host_round(w1t), host_round(w2p), host_round(w2q),
            host_round(w2r))


_NC_CACHE: dict = {}


def _get_nc():
    key = ("main", MODE)
    if key not in _NC_CACHE:
        _NC_CACHE[key] = build()
    return _NC_CACHE[key]


def run(x, w1, w2, trace: bool = False):
    """Shard, run on 8 cores, gather.  Returns (out, BassKernelResults)."""
    x = np.ascontiguousarray(np.asarray(x), dtype=np.float32)
    assert x.shape == (FULL_N, C0, GEOM.h0, GEOM.w0), x.shape
    w1t, w2p, w2q, w2r = pack_weights(w1, w2)
    xs = host_round(x).reshape(N_CORES, GEOM.npc, C0, GEOM.h0, GEOM.w0)
    in_maps = [
        {"x": np.ascontiguousarray(xs[c]), "w1t": w1t, "w2p": w2p,
         "w2q": w2q, "w2r": w2r}
        for c in range(N_CORES)
    ]
    nc = _get_nc()
    res = bass_utils.run_bass_kernel_spmd(
        nc, in_maps, core_ids=list(range(N_CORES)), trace=trace)
    out = np.concatenate([r["out"] for r in res.results], axis=0)
    return out.astype(np.float32), res


def kernel(x, w1, w2):
    out, _ = run(x, w1, w2, trace=False)
    return out


# revision 6
# speedup vs baseline: 1.2113x; 1.2113x over previous
"""Trainium2 Bass/Tile kernel: two chained VALID 3x3 convolutions.

    x  [N,3,256,256] --conv(w1)--> h [N,64,254,254] --conv(w2)--> out [N,128,252,252]

Data-parallel over 8 NeuronCores: batch N=16 -> 2 images per core, conv
weights replicated.  Per core the convs are computed as implicit GEMMs on the
tensor engine.  The kernel is tensor-engine issue-rate bound (the HW activity
monitor clamps the PE to ~1.2 GHz column rate under sustained load), so the
design minimizes total matmul *columns*:

  conv1: contraction over C0*3*3=27 on SBUF partitions (im2col buffer built
         with 9 strided DMAs), one matmul per 2-row output chunk
         (1 column per output pixel -- already minimal).
  conv2: contraction over C1*9=576 done in 5 matmul passes per 2-row chunk
         (vs 9 naive, 6 with one shifted copy):
           H  = [A;  B ]: A = h rows, B = h shifted down 1 row
                -> pairs (0,dj)+(1,dj), dj=0..2, K=128      (3 passes)
           H2 = [D;  A2]: D = h shifted left 1 col, A2 = h copy
                -> pair (2,1)+(2,0) at K=128                (1 pass)
                -> single (2,2) via D shifted one more col, K=64 (1 pass)
         The B/A2/D copies are SBUF->SBUF DMAs off the critical path.
         PSUM accumulates the 5 matmuls, DVE copies to SBUF (bf16),
         DMA to HBM; host casts the bf16 output back to fp32.

MODE selects the matmul dtype:
  "bf16": inputs cast to bfloat16 host-side, fp32 PSUM accumulation
  "tf32": float32r
"""

from contextlib import ExitStack

import ml_dtypes
import numpy as np

import concourse.bass as bass
import concourse.mybir as mybir
import concourse.tile as tile
import concourse.bass_utils as bass_utils
from concourse import bacc

N_CORES = 8
FULL_N = 16
C0, C1, C2 = 3, 64, 128

MODE = "bf16"


def _mm_dt():
    return mybir.dt.bfloat16 if MODE == "bf16" else mybir.dt.float32r


def _np_dt():
    return ml_dtypes.bfloat16 if MODE == "bf16" else np.float32


class Geom:
    def __init__(self, npc, h0, w0, ty):
        self.npc = npc          # images per core
        self.h0, self.w0 = h0, w0
        self.h1, self.w1 = h0 - 2, w0 - 2
        self.h2, self.w2 = h0 - 4, w0 - 4
        self.ty = ty            # conv2 output rows per strip
        assert ty % 2 == 0 and self.h2 % ty == 0


GEOM = Geom(npc=FULL_N // N_CORES, h0=256, w0=256, ty=28)


def _emit(ctx: ExitStack, tc: tile.TileContext, g: Geom, out, x, w1t, w2p, w2q,
          w2r, mm_dt):
    nc = tc.nc
    f32 = mybir.dt.float32
    TY, W1, W2 = g.ty, g.w1, g.w2

    wpool = ctx.enter_context(tc.tile_pool(name="weights", bufs=1))
    b1pool = ctx.enter_context(tc.tile_pool(name="b1", bufs=2))
    hpool = ctx.enter_context(tc.tile_pool(name="h", bufs=2))
    h2pool = ctx.enter_context(tc.tile_pool(name="h2", bufs=2))
    opool = ctx.enter_context(tc.tile_pool(name="o2", bufs=4))
    ps1 = ctx.enter_context(tc.tile_pool(name="ps1", bufs=2, space="PSUM"))
    ps2 = ctx.enter_context(tc.tile_pool(name="ps2", bufs=4, space="PSUM"))

    w1t_sb = wpool.tile([27, C1], mm_dt)
    nc.sync.dma_start(w1t_sb[:], w1t)
    w2p_sb = wpool.tile([128, 3, C2], mm_dt)
    nc.sync.dma_start(w2p_sb[:], w2p)
    w2q_sb = wpool.tile([128, C2], mm_dt)
    nc.sync.dma_start(w2q_sb[:], w2q)
    w2r_sb = wpool.tile([C1, C2], mm_dt)
    nc.sync.dma_start(w2r_sb[:], w2r)

    def conv1(n, y0):
        """Produce the H (=[A;B]) and H2 (=[D;A2]) strips for conv2 rows
        [y0, y0+TY)."""
        # im2col: partition (di*3+dj)*3+c holds x[c, y0+r+di, dj:dj+W1]
        B1 = b1pool.tile([27, TY + 2, W1], mm_dt, tag="b1")
        for t9 in range(9):
            di, dj = divmod(t9, 3)
            nc.sync.dma_start(
                B1[3 * t9:3 * t9 + 3],
                x[n, :, y0 + di:y0 + di + TY + 2, dj:dj + W1])
        H = hpool.tile([128, TY + 2, W1], mm_dt, tag="h")
        H2 = h2pool.tile([128, TY + 2, W1], mm_dt, tag="h2")
        Copy = mybir.ActivationFunctionType.Copy
        for r in range(0, TY + 2, 2):
            P1 = ps1.tile([C1, 2, W1], f32, tag="p1")
            nc.tensor.matmul(P1[:], w1t_sb[:], B1[:, r:r + 2, :],
                             start=True, stop=True)
            # casting copy into partitions 0:64 (h rows r, r+1)
            nc.vector.tensor_copy(H[0:C1, r:r + 2, :], P1[:])
            # B: row-shifted copy into 64:128 (B row r' = h row r'+1),
            # straight from PSUM on the scalar (Act) engine
            if r == 0:
                nc.scalar.activation(H[C1:128, 0:1, :], P1[:, 1:2, :], Copy)
            else:
                nc.scalar.activation(H[C1:128, r - 1:r + 1, :], P1[:], Copy)
            if r >= 2:
                # conv2 only reads H2 rows 2..TY+1
                # A2: plain copy of h into 64:128 on gpsimd (Pool); Pool
                # cannot read PSUM, so source the freshly written A rows
                nc.gpsimd.tensor_copy(H2[C1:128, r:r + 2, :],
                                      H[0:C1, r:r + 2, :])
                # D: col-shifted copy (D col x = h col x+1)
                nc.sync.dma_start(H2[0:C1, r:r + 2, 0:W1 - 1],
                                  H[0:C1, r:r + 2, 1:W1])
        return H, H2

    def conv2(n, y0, H, H2):
        for t in range(0, TY, 2):
            P2 = ps2.tile([C2, 2, W2], f32, tag="p2")
            for dj in range(3):  # pairs: taps (0,dj) + (1,dj), K=128
                nc.tensor.matmul(P2[:], w2p_sb[:, dj, :],
                                 H[:, t:t + 2, dj:dj + W2],
                                 start=(dj == 0), stop=False)
            # pair: taps (2,1) [D] + (2,0) [A2], K=128
            nc.tensor.matmul(P2[:], w2q_sb[:],
                             H2[:, t + 2:t + 4, 0:W2],
                             start=False, stop=False)
            # single: tap (2,2) via D shifted one more col, K=64
            nc.tensor.matmul(P2[:], w2r_sb[:],
                             H2[0:C1, t + 2:t + 4, 1:1 + W2],
                             start=False, stop=True)
            O2 = opool.tile([C2, 2, W2], mm_dt, tag="o2")
            nc.vector.tensor_copy(O2[:], P2[:])
            nc.sync.dma_start(out[n, :, y0 + t:y0 + t + 2, :], O2[:])

    strips = [(n, y0) for n in range(g.npc) for y0 in range(0, g.h2, TY)]
    # software pipeline: emit conv1 of strip s+1 before conv2 of strip s so
    # the tensor engine never waits on the h copies of the strip it consumes
    Hcur = conv1(*strips[0])
    for i, (n, y0) in enumerate(strips):
        Hnext = conv1(*strips[i + 1]) if i + 1 < len(strips) else None
        conv2(n, y0, *Hcur)
        Hcur = Hnext


def build(g: Geom = GEOM, mm_dt=None):
    if mm_dt is None:
        mm_dt = _mm_dt()
    nc = bacc.Bacc("TRN2", target_bir_lowering=False, debug=False,
                   num_devices=N_CORES)
    x = nc.dram_tensor("x", [g.npc, C0, g.h0, g.w0], mm_dt,
                       kind="ExternalInput").ap()
    w1t = nc.dram_tensor("w1t", [27, C1], mm_dt, kind="ExternalInput").ap()
    w2p = nc.dram_tensor("w2p", [128, 3, C2], mm_dt, kind="ExternalInput").ap()
    w2q = nc.dram_tensor("w2q", [128, C2], mm_dt, kind="ExternalInput").ap()
    w2r = nc.dram_tensor("w2r", [C1, C2], mm_dt, kind="ExternalInput").ap()
    out = nc.dram_tensor("out", [g.npc, C2, g.h2, g.w2], mm_dt,
                         kind="ExternalOutput").ap()
    with tile.TileContext(nc) as tc:
        with ExitStack() as ctx:
            _emit(ctx, tc, g, out, x, w1t, w2p, w2q, w2r, mm_dt)
    nc.compile()
    return nc


def host_round(a: np.ndarray) -> np.ndarray:
    """Cast fp32 to the matmul storage dtype (bf16 cast, or tf32 rounding)."""
    a = np.ascontiguousarray(a, dtype=np.float32)
    if MODE == "bf16":
        return a.astype(ml_dtypes.bfloat16)
    b = a.view(np.uint32).copy()
    b += 0xFFF + ((b >> 13) & 1)
    b &= np.uint32(0xFFFFE000)
    return b.view(np.float32)


def pack_weights(w1: np.ndarray, w2: np.ndarray):
    """Host-side repack so every device DMA is contiguous.

    w1t[p, o] = w1[o, c, di, dj] with p = (di*3+dj)*3 + c  (matches im2col)
    w2p[k, dj, o]: k<64 -> w2[o, k, 0, dj];  k>=64 -> w2[o, k-64, 1, dj]
    w2q[k, o]:     k<64 -> w2[o, k, 2, 1];   k>=64 -> w2[o, k-64, 2, 0]
    w2r[c, o] = w2[o, c, 2, 2]
    """
    w1 = np.ascontiguousarray(np.asarray(w1), dtype=np.float32)
    w2 = np.ascontiguousarray(np.asarray(w2), dtype=np.float32)
    w1t = np.ascontiguousarray(w1.transpose(2, 3, 1, 0).reshape(27, C1))
    w2p = np.empty((128, 3, C2), np.float32)
    w2p[:C1] = w2[:, :, 0, :].transpose(1, 2, 0)
    w2p[C1:] = w2[:, :, 1, :].transpose(1, 2, 0)
    w2q = np.empty((128, C2), np.float32)
    w2q[:C1] = w2[:, :, 2, 1].transpose(1, 0)
    w2q[C1:] = w2[:, :, 2, 0].transpose(1, 0)
    w2r = np.ascontiguousarray(w2[:, :, 2, 2].transpose(1, 0))
    return (host_round(w1t), host_round(w2p), host_round(w2q),
            host_round(w2r))


_NC_CACHE: dict = {}


def _get_nc():
    key = ("main", MODE)
    if key not in _NC_CACHE:
        _NC_CACHE[key] = build()
    return _NC_CACHE[key]


def run(x, w1, w2, trace: bool = False):
    """Shard, run on 8 cores, gather.  Returns (out, BassKernelResults)."""
    x = np.ascontiguousarray(np.asarray(x), dtype=np.float32)
    assert x.shape == (FULL_N, C0, GEOM.h0, GEOM.w0), x.shape
    w1t, w2p, w2q, w2r = pack_weights(w1, w2)
    xs = host_round(x).reshape(N_CORES, GEOM.npc, C0, GEOM.h0, GEOM.w0)
    in_maps = [
        {"x": np.ascontiguousarray(xs[c]), "w1t": w1t, "w2p": w2p,
         "w2q": w2q, "w2r": w2r}
        for c in range(N_CORES)
    ]
    nc = _get_nc()
    res = bass_utils.run_bass_kernel_spmd(
        nc, in_maps, core_ids=list(range(N_CORES)), trace=trace)
    out = np.concatenate([r["out"] for r in res.results], axis=0)
    return out.astype(np.float32), res


def kernel(x, w1, w2):
    out, _ = run(x, w1, w2, trace=False)
    return out


# revision 7
# speedup vs baseline: 1.3454x; 1.1107x over previous
"""Trainium2 Bass/Tile kernel: two chained VALID 3x3 convolutions.

    x  [N,3,256,256] --conv(w1)--> h [N,64,254,254] --conv(w2)--> out [N,128,252,252]

Data-parallel over 8 NeuronCores: batch N=16 -> 2 images per core, conv
weights replicated.  Per core the convs are computed as implicit GEMMs on the
tensor engine.  The kernel is tensor-engine issue-rate bound (the HW activity
monitor clamps the PE to ~1.2 GHz column rate under sustained load), so the
design minimizes total matmul *columns*:

  conv1: contraction over C0*3*3=27 on SBUF partitions (im2col buffer built
         with 9 strided DMAs), one matmul per 2-row output chunk
         (1 column per output pixel -- already minimal).
  conv2: contraction over C1*9=576 done in 5 matmul passes per 2-row chunk
         (vs 9 naive, 6 with one shifted copy):
           H  = [A;  B ]: A = h rows, B = h shifted down 1 row
                -> pairs (0,dj)+(1,dj), dj=0..2, K=128      (3 passes)
           H2 = [D;  A2]: D = h shifted left 1 col, A2 = h copy
                -> pair (2,1)+(2,0) at K=128                (1 pass)
                -> single (2,2) via D shifted one more col, K=64 (1 pass)
         The B/A2/D copies are SBUF->SBUF DMAs off the critical path.
         PSUM accumulates the 5 matmuls, DVE copies to SBUF (bf16),
         DMA to HBM; host casts the bf16 output back to fp32.

MODE selects the matmul dtype:
  "bf16": inputs cast to bfloat16 host-side, fp32 PSUM accumulation
  "tf32": float32r
"""

from contextlib import ExitStack

import ml_dtypes
import numpy as np

import concourse.bass as bass
import concourse.mybir as mybir
import concourse.tile as tile
import concourse.bass_utils as bass_utils
from concourse import bacc

N_CORES = 8
FULL_N = 16
C0, C1, C2 = 3, 64, 128

MODE = "bf16"


def _mm_dt():
    return mybir.dt.bfloat16 if MODE == "bf16" else mybir.dt.float32r


def _np_dt():
    return ml_dtypes.bfloat16 if MODE == "bf16" else np.float32


class Geom:
    def __init__(self, npc, h0, w0, ty):
        self.npc = npc          # images per core
        self.h0, self.w0 = h0, w0
        self.h1, self.w1 = h0 - 2, w0 - 2
        self.h2, self.w2 = h0 - 4, w0 - 4
        self.ty = ty            # conv2 output rows per strip
        assert ty % 2 == 0 and self.h2 % ty == 0


GEOM = Geom(npc=FULL_N // N_CORES, h0=256, w0=256, ty=28)


def _emit(ctx: ExitStack, tc: tile.TileContext, g: Geom, out, x, w1t, w2p, w2q,
          w2r, mm_dt):
    nc = tc.nc
    f32 = mybir.dt.float32
    Copy = mybir.ActivationFunctionType.Copy
    TY, W1, W2 = g.ty, g.w1, g.w2

    wpool = ctx.enter_context(tc.tile_pool(name="weights", bufs=1))
    b1pool = ctx.enter_context(tc.tile_pool(name="b1", bufs=2))
    hpool = ctx.enter_context(tc.tile_pool(name="h", bufs=2))
    h2pool = ctx.enter_context(tc.tile_pool(name="h2", bufs=2))
    opool = ctx.enter_context(tc.tile_pool(name="o2", bufs=4))
    ps1 = ctx.enter_context(tc.tile_pool(name="ps1", bufs=3, space="PSUM"))
    ps2 = ctx.enter_context(tc.tile_pool(name="ps2", bufs=4, space="PSUM"))

    w1t_sb = wpool.tile([27, C1], mm_dt)
    nc.sync.dma_start(w1t_sb[:], w1t)
    w2p_sb = wpool.tile([128, 3, C2], mm_dt)
    nc.sync.dma_start(w2p_sb[:], w2p)
    w2q_sb = wpool.tile([128, C2], mm_dt)
    nc.sync.dma_start(w2q_sb[:], w2q)
    w2r_sb = wpool.tile([C1, C2], mm_dt)
    nc.sync.dma_start(w2r_sb[:], w2r)

    def im2col(n, y0):
        """Allocate B1 for a strip; return thunks that emit its 9 DMAs."""
        B1 = b1pool.tile([27, TY + 2, W1], mm_dt, tag="b1")

        def dma(t9):
            di, dj = divmod(t9, 3)
            nc.sync.dma_start(
                B1[3 * t9:3 * t9 + 3],
                x[n, :, y0 + di:y0 + di + TY + 2, dj:dj + W1])
        return B1, [lambda t9=t9: dma(t9) for t9 in range(9)]

    def conv1_alloc():
        H = hpool.tile([128, TY + 2, W1], mm_dt, tag="h")
        H2 = h2pool.tile([128, TY + 2, W1], mm_dt, tag="h2")
        return H, H2

    def conv1_chunk(B1, H, H2, r):
        """h rows r, r+1: matmul + A (DVE), B/A2 (scalar), D (DMA)."""
        P1 = ps1.tile([C1, 2, W1], f32, tag="p1")
        nc.tensor.matmul(P1[:], w1t_sb[:], B1[:, r:r + 2, :],
                         start=True, stop=True)
        # A: h rows into partitions 0:64 (casting copy)
        nc.vector.tensor_copy(H[0:C1, r:r + 2, :], P1[:])
        # B: row-shifted copy into 64:128 (B row r' = h row r'+1), from PSUM
        if r == 0:
            nc.scalar.activation(H[C1:128, 0:1, :], P1[:, 1:2, :], Copy)
        else:
            nc.scalar.activation(H[C1:128, r - 1:r + 1, :], P1[:], Copy)
        if r >= 2:
            # conv2 only reads H2 rows 2..TY+1
            # A2: plain copy of h into 64:128, from PSUM
            nc.scalar.activation(H2[C1:128, r:r + 2, :], P1[:], Copy)
            # D: col-shifted copy (D col x = h col x+1)
            nc.sync.dma_start(H2[0:C1, r:r + 2, 0:W1 - 1],
                              H[0:C1, r:r + 2, 1:W1])

    def conv2_chunk(n, y0, H, H2, t):
        P2 = ps2.tile([C2, 2, W2], f32, tag="p2")
        for dj in range(3):  # pairs: taps (0,dj) + (1,dj), K=128
            nc.tensor.matmul(P2[:], w2p_sb[:, dj, :],
                             H[:, t:t + 2, dj:dj + W2],
                             start=(dj == 0), stop=False)
        # pair: taps (2,1) [D] + (2,0) [A2], K=128
        nc.tensor.matmul(P2[:], w2q_sb[:],
                         H2[:, t + 2:t + 4, 0:W2],
                         start=False, stop=False)
        # single: tap (2,2) via D shifted one more col, K=64
        nc.tensor.matmul(P2[:], w2r_sb[:],
                         H2[0:C1, t + 2:t + 4, 1:1 + W2],
                         start=False, stop=True)
        O2 = opool.tile([C2, 2, W2], mm_dt, tag="o2")
        nc.vector.tensor_copy(O2[:], P2[:])
        nc.sync.dma_start(out[n, :, y0 + t:y0 + t + 2, :], O2[:])

    strips = [(n, y0) for n in range(g.npc) for y0 in range(0, g.h2, TY)]
    ns = len(strips)

    # prologue: load strip 0, run conv1(0) as a burst, start loading strip 1
    B1_0, dmas = im2col(*strips[0])
    for t in dmas:
        t()
    cur = conv1_alloc()
    for r in range(0, TY + 2, 2):
        conv1_chunk(B1_0, *cur, r)
    B1s = {1: None}
    if ns > 1:
        B1_1, dmas = im2col(*strips[1])
        for t in dmas:
            t()
        B1s[1] = B1_1

    # steady state: conv2(i) interleaved with conv1(i+1) and im2col(i+2).
    # Emission order == tensor-engine execution order, so spreading the
    # conv1 matmuls between conv2 chunks keeps every producer (DVE/scalar/
    # DMA) ahead of its consumer without bursts.
    for i in range(ns):
        n, y0 = strips[i]
        c1work = []
        nxt = None
        if i + 1 < ns:
            nxt = conv1_alloc()
            B1n = B1s.pop(i + 1)
            c1work = [lambda r=r, B1n=B1n, nxt=nxt: conv1_chunk(B1n, *nxt, r)
                      for r in range(0, TY + 2, 2)]
        imwork = []
        if i + 2 < ns:
            B1x, imwork = im2col(*strips[i + 2])
            B1s[i + 2] = B1x
        c2work = [lambda t=t: conv2_chunk(n, y0, *cur, t)
                  for t in range(0, TY, 2)]

        for t in range(len(c2work)):
            if t < len(c1work):
                c1work[t]()
            c2work[t]()
            if t < len(imwork):
                imwork[t]()
        for w in c1work[len(c2work):]:
            w()
        cur = nxt


def build(g: Geom = GEOM, mm_dt=None):
    if mm_dt is None:
        mm_dt = _mm_dt()
    nc = bacc.Bacc("TRN2", target_bir_lowering=False, debug=False,
                   num_devices=N_CORES)
    x = nc.dram_tensor("x", [g.npc, C0, g.h0, g.w0], mm_dt,
                       kind="ExternalInput").ap()
    w1t = nc.dram_tensor("w1t", [27, C1], mm_dt, kind="ExternalInput").ap()
    w2p = nc.dram_tensor("w2p", [128, 3, C2], mm_dt, kind="ExternalInput").ap()
    w2q = nc.dram_tensor("w2q", [128, C2], mm_dt, kind="ExternalInput").ap()
    w2r = nc.dram_tensor("w2r", [C1, C2], mm_dt, kind="ExternalInput").ap()
    out = nc.dram_tensor("out", [g.npc, C2, g.h2, g.w2], mm_dt,
                         kind="ExternalOutput").ap()
    with tile.TileContext(nc) as tc:
        with ExitStack() as ctx:
            _emit(ctx, tc, g, out, x, w1t, w2p, w2q, w2r, mm_dt)
    nc.compile()
    return nc


def host_round(a: np.ndarray) -> np.ndarray:
    """Cast fp32 to the matmul storage dtype (bf16 cast, or tf32 rounding)."""
    a = np.ascontiguousarray(a, dtype=np.float32)
    if MODE == "bf16":
        return a.astype(ml_dtypes.bfloat16)
    b = a.view(np.uint32).copy()
    b += 0xFFF + ((b >> 13) & 1)
    b &= np.uint32(0xFFFFE000)
    return b.view(np.float32)


def pack_weights(w1: np.ndarray, w2: np.ndarray):
    """Host-side repack so every device DMA is contiguous.

    w1t[p, o] = w1[o, c, di, dj] with p = (di*3+dj)*3 + c  (matches im2col)
    w2p[k, dj, o]: k<64 -> w2[o, k, 0, dj];  k>=64 -> w2[o, k-64, 1, dj]
    w2q[k, o]:     k<64 -> w2[o, k, 2, 1];   k>=64 -> w2[o, k-64, 2, 0]
    w2r[c, o] = w2[o, c, 2, 2]
    """
    w1 = np.ascontiguousarray(np.asarray(w1), dtype=np.float32)
    w2 = np.ascontiguousarray(np.asarray(w2), dtype=np.float32)
    w1t = np.ascontiguousarray(w1.transpose(2, 3, 1, 0).reshape(27, C1))
    w2p = np.empty((128, 3, C2), np.float32)
    w2p[:C1] = w2[:, :, 0, :].transpose(1, 2, 0)
    w2p[C1:] = w2[:, :, 1, :].transpose(1, 2, 0)
    w2q = np.empty((128, C2), np.float32)
    w2q[:C1] = w2[:, :, 2, 1].transpose(1, 0)
    w2q[C1:] = w2[:, :, 2, 0].transpose(1, 0)
    w2r = np.ascontiguousarray(w2[:, :, 2, 2].transpose(1, 0))
    return (host_round(w1t), host_round(w2p), host_round(w2q),
            host_round(w2r))


_NC_CACHE: dict = {}


def _get_nc():
    key = ("main", MODE)
    if key not in _NC_CACHE:
        _NC_CACHE[key] = build()
    return _NC_CACHE[key]


def run(x, w1, w2, trace: bool = False):
    """Shard, run on 8 cores, gather.  Returns (out, BassKernelResults)."""
    x = np.ascontiguousarray(np.asarray(x), dtype=np.float32)
    assert x.shape == (FULL_N, C0, GEOM.h0, GEOM.w0), x.shape
    w1t, w2p, w2q, w2r = pack_weights(w1, w2)
    xs = host_round(x).reshape(N_CORES, GEOM.npc, C0, GEOM.h0, GEOM.w0)
    in_maps = [
        {"x": np.ascontiguousarray(xs[c]), "w1t": w1t, "w2p": w2p,
         "w2q": w2q, "w2r": w2r}
        for c in range(N_CORES)
    ]
    nc = _get_nc()
    res = bass_utils.run_bass_kernel_spmd(
        nc, in_maps, core_ids=list(range(N_CORES)), trace=trace)
    out = np.concatenate([r["out"] for r in res.results], axis=0)
    return out.astype(np.float32), res


def kernel(x, w1, w2):
    out, _ = run(x, w1, w2, trace=False)
    return out


# revision 8
# speedup vs baseline: 1.3545x; 1.0067x over previous
"""Trainium2 Bass/Tile kernel: two chained VALID 3x3 convolutions.

    x  [N,3,256,256] --conv(w1)--> h [N,64,254,254] --conv(w2)--> out [N,128,252,252]

Data-parallel over 8 NeuronCores: batch N=16 -> 2 images per core, conv
weights replicated.  Per core the convs are computed as implicit GEMMs on the
tensor engine.  The kernel is tensor-engine issue-rate bound (the HW activity
monitor clamps the PE to ~1.2 GHz column rate under sustained load), so the
design minimizes total matmul *columns*:

  conv1: contraction over C0*3*3=27 on SBUF partitions (im2col buffer built
         with 9 strided DMAs), one matmul per 2-row output chunk
         (1 column per output pixel -- already minimal).
  conv2: contraction over C1*9=576 done in 5 matmul passes per 2-row chunk
         (vs 9 naive, 6 with one shifted copy):
           H  = [A;  B ]: A = h rows, B = h shifted down 1 row
                -> pairs (0,dj)+(1,dj), dj=0..2, K=128      (3 passes)
           H2 = [D;  A2]: D = h shifted left 1 col, A2 = h copy
                -> pair (2,1)+(2,0) at K=128                (1 pass)
                -> single (2,2) via D shifted one more col, K=64 (1 pass)
         The B/A2/D copies are SBUF->SBUF DMAs off the critical path.
         PSUM accumulates the 5 matmuls, DVE copies to SBUF (bf16),
         DMA to HBM; host casts the bf16 output back to fp32.

MODE selects the matmul dtype:
  "bf16": inputs cast to bfloat16 host-side, fp32 PSUM accumulation
  "tf32": float32r
"""

from contextlib import ExitStack

import ml_dtypes
import numpy as np

import concourse.bass as bass
import concourse.mybir as mybir
import concourse.tile as tile
import concourse.bass_utils as bass_utils
from concourse import bacc

N_CORES = 8
FULL_N = 16
C0, C1, C2 = 3, 64, 128

MODE = "bf16"


def _mm_dt():
    return mybir.dt.bfloat16 if MODE == "bf16" else mybir.dt.float32r


def _np_dt():
    return ml_dtypes.bfloat16 if MODE == "bf16" else np.float32


class Geom:
    def __init__(self, npc, h0, w0, ty):
        self.npc = npc          # images per core
        self.h0, self.w0 = h0, w0
        self.h1, self.w1 = h0 - 2, w0 - 2
        self.h2, self.w2 = h0 - 4, w0 - 4
        self.ty = ty            # conv2 output rows per strip
        assert ty % 2 == 0 and self.h2 % ty == 0


GEOM = Geom(npc=FULL_N // N_CORES, h0=256, w0=256, ty=28)


def _emit(ctx: ExitStack, tc: tile.TileContext, g: Geom, out, x, w1t, w2p, w2q,
          w2r, mm_dt):
    nc = tc.nc
    f32 = mybir.dt.float32
    Copy = mybir.ActivationFunctionType.Copy
    TY, W1, W2 = g.ty, g.w1, g.w2

    wpool = ctx.enter_context(tc.tile_pool(name="weights", bufs=1))
    b1pool = ctx.enter_context(tc.tile_pool(name="b1", bufs=2))
    hpool = ctx.enter_context(tc.tile_pool(name="h", bufs=2))
    h2pool = ctx.enter_context(tc.tile_pool(name="h2", bufs=2))
    opool = ctx.enter_context(tc.tile_pool(name="o2", bufs=6))
    ps1 = ctx.enter_context(tc.tile_pool(name="ps1", bufs=4, space="PSUM"))
    ps2 = ctx.enter_context(tc.tile_pool(name="ps2", bufs=4, space="PSUM"))

    w1t_sb = wpool.tile([27, C1], mm_dt)
    nc.sync.dma_start(w1t_sb[:], w1t)
    w2p_sb = wpool.tile([128, 3, C2], mm_dt)
    nc.sync.dma_start(w2p_sb[:], w2p)
    w2q_sb = wpool.tile([128, C2], mm_dt)
    nc.sync.dma_start(w2q_sb[:], w2q)
    w2r_sb = wpool.tile([C1, C2], mm_dt)
    nc.sync.dma_start(w2r_sb[:], w2r)

    def im2col(n, y0):
        """Allocate B1 for a strip; return thunks that emit its 9 DMAs."""
        B1 = b1pool.tile([27, TY + 2, W1], mm_dt, tag="b1")

        def dma(t9):
            di, dj = divmod(t9, 3)
            nc.sync.dma_start(
                B1[3 * t9:3 * t9 + 3],
                x[n, :, y0 + di:y0 + di + TY + 2, dj:dj + W1])
        return B1, [lambda t9=t9: dma(t9) for t9 in range(9)]

    def conv1_alloc():
        H = hpool.tile([128, TY + 2, W1], mm_dt, tag="h")
        H2 = h2pool.tile([128, TY + 2, W1], mm_dt, tag="h2")
        return H, H2

    def conv1_chunk(B1, H, H2, r):
        """h rows r, r+1: matmul + A (DVE), B/A2 (scalar), D (DMA)."""
        P1 = ps1.tile([C1, 2, W1], f32, tag="p1")
        nc.tensor.matmul(P1[:], w1t_sb[:], B1[:, r:r + 2, :],
                         start=True, stop=True)
        # A: h rows into partitions 0:64 (casting copy)
        nc.vector.tensor_copy(H[0:C1, r:r + 2, :], P1[:])
        # B: row-shifted copy into 64:128 (B row r' = h row r'+1), from PSUM
        if r == 0:
            nc.scalar.activation(H[C1:128, 0:1, :], P1[:, 1:2, :], Copy)
        else:
            nc.scalar.activation(H[C1:128, r - 1:r + 1, :], P1[:], Copy)
        if r >= 2:
            # conv2 only reads H2 rows 2..TY+1
            # A2: plain copy of h into 64:128 (DMA, SBUF->SBUF)
            nc.sync.dma_start(H2[C1:128, r:r + 2, :], H[0:C1, r:r + 2, :])
            # D: col-shifted copy (D col x = h col x+1)
            nc.sync.dma_start(H2[0:C1, r:r + 2, 0:W1 - 1],
                              H[0:C1, r:r + 2, 1:W1])

    def conv2_chunk(n, y0, H, H2, t):
        P2 = ps2.tile([C2, 2, W2], f32, tag="p2")
        for dj in range(3):  # pairs: taps (0,dj) + (1,dj), K=128
            nc.tensor.matmul(P2[:], w2p_sb[:, dj, :],
                             H[:, t:t + 2, dj:dj + W2],
                             start=(dj == 0), stop=False)
        # pair: taps (2,1) [D] + (2,0) [A2], K=128
        nc.tensor.matmul(P2[:], w2q_sb[:],
                         H2[:, t + 2:t + 4, 0:W2],
                         start=False, stop=False)
        # single: tap (2,2) via D shifted one more col, K=64
        nc.tensor.matmul(P2[:], w2r_sb[:],
                         H2[0:C1, t + 2:t + 4, 1:1 + W2],
                         start=False, stop=True)
        O2 = opool.tile([C2, 2, W2], mm_dt, tag="o2")
        # alternate the PSUM->SBUF out-cast between DVE and scalar so that
        # neither falls behind the boosted (k=8) tensor cadence
        if (t // 2) % 2 == 0:
            nc.vector.tensor_copy(O2[:], P2[:])
        else:
            nc.scalar.activation(O2[:], P2[:], Copy)
        nc.sync.dma_start(out[n, :, y0 + t:y0 + t + 2, :], O2[:])

    strips = [(n, y0) for n in range(g.npc) for y0 in range(0, g.h2, TY)]
    ns = len(strips)

    # prologue: load strip 0, run conv1(0) as a burst, start loading strip 1
    B1_0, dmas = im2col(*strips[0])
    for t in dmas:
        t()
    cur = conv1_alloc()
    for r in range(0, TY + 2, 2):
        conv1_chunk(B1_0, *cur, r)
    B1s = {1: None}
    if ns > 1:
        B1_1, dmas = im2col(*strips[1])
        for t in dmas:
            t()
        B1s[1] = B1_1

    # steady state: conv2(i) interleaved with conv1(i+1) and im2col(i+2).
    # Emission order == tensor-engine execution order, so spreading the
    # conv1 matmuls between conv2 chunks keeps every producer (DVE/scalar/
    # DMA) ahead of its consumer without bursts.
    for i in range(ns):
        n, y0 = strips[i]
        c1work = []
        nxt = None
        if i + 1 < ns:
            nxt = conv1_alloc()
            B1n = B1s.pop(i + 1)
            c1work = [lambda r=r, B1n=B1n, nxt=nxt: conv1_chunk(B1n, *nxt, r)
                      for r in range(0, TY + 2, 2)]
        imwork = []
        if i + 2 < ns:
            B1x, imwork = im2col(*strips[i + 2])
            B1s[i + 2] = B1x
        c2work = [lambda t=t: conv2_chunk(n, y0, *cur, t)
                  for t in range(0, TY, 2)]

        for t in range(len(c2work)):
            if t < len(c1work):
                c1work[t]()
            c2work[t]()
            if t < len(imwork):
                imwork[t]()
        for w in c1work[len(c2work):]:
            w()
        cur = nxt


def build(g: Geom = GEOM, mm_dt=None):
    if mm_dt is None:
        mm_dt = _mm_dt()
    nc = bacc.Bacc("TRN2", target_bir_lowering=False, debug=False,
                   num_devices=N_CORES)
    x = nc.dram_tensor("x", [g.npc, C0, g.h0, g.w0], mm_dt,
                       kind="ExternalInput").ap()
    w1t = nc.dram_tensor("w1t", [27, C1], mm_dt, kind="ExternalInput").ap()
    w2p = nc.dram_tensor("w2p", [128, 3, C2], mm_dt, kind="ExternalInput").ap()
    w2q = nc.dram_tensor("w2q", [128, C2], mm_dt, kind="ExternalInput").ap()
    w2r = nc.dram_tensor("w2r", [C1, C2], mm_dt, kind="ExternalInput").ap()
    out = nc.dram_tensor("out", [g.npc, C2, g.h2, g.w2], mm_dt,
                         kind="ExternalOutput").ap()
    with tile.TileContext(nc) as tc:
        with ExitStack() as ctx:
            _emit(ctx, tc, g, out, x, w1t, w2p, w2q, w2r, mm_dt)
    nc.compile()
    return nc


def host_round(a: np.ndarray) -> np.ndarray:
    """Cast fp32 to the matmul storage dtype (bf16 cast, or tf32 rounding)."""
    a = np.ascontiguousarray(a, dtype=np.float32)
    if MODE == "bf16":
        return a.astype(ml_dtypes.bfloat16)
    b = a.view(np.uint32).copy()
    b += 0xFFF + ((b >> 13) & 1)
    b &= np.uint32(0xFFFFE000)
    return b.view(np.float32)


def pack_weights(w1: np.ndarray, w2: np.ndarray):
    """Host-side repack so every device DMA is contiguous.

    w1t[p, o] = w1[o, c, di, dj] with p = (di*3+dj)*3 + c  (matches im2col)
    w2p[k, dj, o]: k<64 -> w2[o, k, 0, dj];  k>=64 -> w2[o, k-64, 1, dj]
    w2q[k, o]:     k<64 -> w2[o, k, 2, 1];   k>=64 -> w2[o, k-64, 2, 0]
    w2r[c, o] = w2[o, c, 2, 2]
    """
    w1 = np.ascontiguousarray(np.asarray(w1), dtype=np.float32)
    w2 = np.ascontiguousarray(np.asarray(w2), dtype=np.float32)
    w1t = np.ascontiguousarray(w1.transpose(2, 3, 1, 0).reshape(27, C1))
    w2p = np.empty((128, 3, C2), np.float32)
    w2p[:C1] = w2[:, :, 0, :].transpose(1, 2, 0)
    w2p[C1:] = w2[:, :, 1, :].transpose(1, 2, 0)
    w2q = np.empty((128, C2), np.float32)
    w2q[:C1] = w2[:, :, 2, 1].transpose(1, 0)
    w2q[C1:] = w2[:, :, 2, 0].transpose(1, 0)
    w2r = np.ascontiguousarray(w2[:, :, 2, 2].transpose(1, 0))
    return (host_round(w1t), host_round(w2p), host_round(w2q),
            host_round(w2r))


_NC_CACHE: dict = {}


def _get_nc():
    key = ("main", MODE)
    if key not in _NC_CACHE:
        _NC_CACHE[key] = build()
    return _NC_CACHE[key]


def run(x, w1, w2, trace: bool = False):
    """Shard, run on 8 cores, gather.  Returns (out, BassKernelResults)."""
    x = np.ascontiguousarray(np.asarray(x), dtype=np.float32)
    assert x.shape == (FULL_N, C0, GEOM.h0, GEOM.w0), x.shape
    w1t, w2p, w2q, w2r = pack_weights(w1, w2)
    xs = host_round(x).reshape(N_CORES, GEOM.npc, C0, GEOM.h0, GEOM.w0)
    in_maps = [
        {"x": np.ascontiguousarray(xs[c]), "w1t": w1t, "w2p": w2p,
         "w2q": w2q, "w2r": w2r}
        for c in range(N_CORES)
    ]
    nc = _get_nc()
    res = bass_utils.run_bass_kernel_spmd(
        nc, in_maps, core_ids=list(range(N_CORES)), trace=trace)
    out = np.concatenate([r["out"] for r in res.results], axis=0)
    return out.astype(np.float32), res


def kernel(x, w1, w2):
    out, _ = run(x, w1, w2, trace=False)
    return out


# revision 9
# speedup vs baseline: 1.3577x; 1.0024x over previous
"""Trainium2 Bass/Tile kernel: two chained VALID 3x3 convolutions.

    x  [N,3,256,256] --conv(w1)--> h [N,64,254,254] --conv(w2)--> out [N,128,252,252]

Data-parallel over 8 NeuronCores: batch N=16 -> 2 images per core, conv
weights replicated.  Per core the convs are computed as implicit GEMMs on the
tensor engine.  The kernel is tensor-engine issue-rate bound (the HW activity
monitor clamps the PE to ~1.2 GHz column rate under sustained load), so the
design minimizes total matmul *columns*:

  conv1: contraction over C0*3*3=27 on SBUF partitions (im2col buffer built
         with 9 strided DMAs), one matmul per 2-row output chunk
         (1 column per output pixel -- already minimal).
  conv2: contraction over C1*9=576 done in 5 matmul passes per 2-row chunk
         (vs 9 naive, 6 with one shifted copy):
           H  = [A;  B ]: A = h rows, B = h shifted down 1 row
                -> pairs (0,dj)+(1,dj), dj=0..2, K=128      (3 passes)
           H2 = [D;  A2]: D = h shifted left 1 col, A2 = h copy
                -> pair (2,1)+(2,0) at K=128                (1 pass)
                -> single (2,2) via D shifted one more col, K=64 (1 pass)
         The B/A2/D copies are SBUF->SBUF DMAs off the critical path.
         PSUM accumulates the 5 matmuls, DVE copies to SBUF (bf16),
         DMA to HBM; host casts the bf16 output back to fp32.

MODE selects the matmul dtype:
  "bf16": inputs cast to bfloat16 host-side, fp32 PSUM accumulation
  "tf32": float32r
"""

from contextlib import ExitStack

import ml_dtypes
import numpy as np

import concourse.bass as bass
import concourse.mybir as mybir
import concourse.tile as tile
import concourse.bass_utils as bass_utils
from concourse import bacc

N_CORES = 8
FULL_N = 16
C0, C1, C2 = 3, 64, 128

MODE = "bf16"


def _mm_dt():
    return mybir.dt.bfloat16 if MODE == "bf16" else mybir.dt.float32r


def _np_dt():
    return ml_dtypes.bfloat16 if MODE == "bf16" else np.float32


class Geom:
    def __init__(self, npc, h0, w0, ty):
        self.npc = npc          # images per core
        self.h0, self.w0 = h0, w0
        self.h1, self.w1 = h0 - 2, w0 - 2
        self.h2, self.w2 = h0 - 4, w0 - 4
        self.ty = ty            # conv2 output rows per strip
        assert ty % 2 == 0 and self.h2 % ty == 0


GEOM = Geom(npc=FULL_N // N_CORES, h0=256, w0=256, ty=42)


def _emit(ctx: ExitStack, tc: tile.TileContext, g: Geom, out, x, w1t, w2p, w2q,
          w2r, mm_dt):
    nc = tc.nc
    f32 = mybir.dt.float32
    Copy = mybir.ActivationFunctionType.Copy
    TY, W1, W2 = g.ty, g.w1, g.w2

    wpool = ctx.enter_context(tc.tile_pool(name="weights", bufs=1))
    b1pool = ctx.enter_context(tc.tile_pool(name="b1", bufs=2))
    hpool = ctx.enter_context(tc.tile_pool(name="h", bufs=2))
    h2pool = ctx.enter_context(tc.tile_pool(name="h2", bufs=2))
    opool = ctx.enter_context(tc.tile_pool(name="o2", bufs=6))
    ps1 = ctx.enter_context(tc.tile_pool(name="ps1", bufs=4, space="PSUM"))
    ps2 = ctx.enter_context(tc.tile_pool(name="ps2", bufs=4, space="PSUM"))

    w1t_sb = wpool.tile([27, C1], mm_dt)
    nc.sync.dma_start(w1t_sb[:], w1t)
    w2p_sb = wpool.tile([128, 3, C2], mm_dt)
    nc.sync.dma_start(w2p_sb[:], w2p)
    w2q_sb = wpool.tile([128, C2], mm_dt)
    nc.sync.dma_start(w2q_sb[:], w2q)
    w2r_sb = wpool.tile([C1, C2], mm_dt)
    nc.sync.dma_start(w2r_sb[:], w2r)

    def im2col(n, y0):
        """Allocate B1 for a strip; return thunks that emit its 9 DMAs."""
        B1 = b1pool.tile([27, TY + 2, W1], mm_dt, tag="b1")

        def dma(t9):
            di, dj = divmod(t9, 3)
            nc.sync.dma_start(
                B1[3 * t9:3 * t9 + 3],
                x[n, :, y0 + di:y0 + di + TY + 2, dj:dj + W1])
        return B1, [lambda t9=t9: dma(t9) for t9 in range(9)]

    def conv1_alloc():
        H = hpool.tile([128, TY + 2, W1], mm_dt, tag="h")
        H2 = h2pool.tile([128, TY + 2, W1], mm_dt, tag="h2")
        return H, H2

    def conv1_chunk(B1, H, H2, r):
        """h rows r, r+1: matmul + A (DVE), B/A2 (scalar), D (DMA)."""
        P1 = ps1.tile([C1, 2, W1], f32, tag="p1")
        nc.tensor.matmul(P1[:], w1t_sb[:], B1[:, r:r + 2, :],
                         start=True, stop=True)
        # A: h rows into partitions 0:64 (casting copy)
        nc.vector.tensor_copy(H[0:C1, r:r + 2, :], P1[:])
        # B: row-shifted copy into 64:128 (B row r' = h row r'+1), from PSUM
        if r == 0:
            nc.scalar.activation(H[C1:128, 0:1, :], P1[:, 1:2, :], Copy)
        else:
            nc.scalar.activation(H[C1:128, r - 1:r + 1, :], P1[:], Copy)
        if r >= 2:
            # conv2 only reads H2 rows 2..TY+1
            # A2: plain copy of h into 64:128 (DMA, SBUF->SBUF)
            nc.sync.dma_start(H2[C1:128, r:r + 2, :], H[0:C1, r:r + 2, :])
            # D: col-shifted copy (D col x = h col x+1)
            nc.sync.dma_start(H2[0:C1, r:r + 2, 0:W1 - 1],
                              H[0:C1, r:r + 2, 1:W1])

    def conv2_chunk(n, y0, H, H2, t):
        P2 = ps2.tile([C2, 2, W2], f32, tag="p2")
        for dj in range(3):  # pairs: taps (0,dj) + (1,dj), K=128
            nc.tensor.matmul(P2[:], w2p_sb[:, dj, :],
                             H[:, t:t + 2, dj:dj + W2],
                             start=(dj == 0), stop=False)
        # pair: taps (2,1) [D] + (2,0) [A2], K=128
        nc.tensor.matmul(P2[:], w2q_sb[:],
                         H2[:, t + 2:t + 4, 0:W2],
                         start=False, stop=False)
        # single: tap (2,2) via D shifted one more col, K=64
        nc.tensor.matmul(P2[:], w2r_sb[:],
                         H2[0:C1, t + 2:t + 4, 1:1 + W2],
                         start=False, stop=True)
        O2 = opool.tile([C2, 2, W2], mm_dt, tag="o2")
        # alternate the PSUM->SBUF out-cast between DVE and scalar so that
        # neither falls behind the boosted (k=8) tensor cadence
        if (t // 2) % 2 == 0:
            nc.vector.tensor_copy(O2[:], P2[:])
        else:
            nc.scalar.activation(O2[:], P2[:], Copy)
        nc.sync.dma_start(out[n, :, y0 + t:y0 + t + 2, :], O2[:])

    strips = [(n, y0) for n in range(g.npc) for y0 in range(0, g.h2, TY)]
    ns = len(strips)

    # prologue: load strip 0, run conv1(0) as a burst, start loading strip 1
    B1_0, dmas = im2col(*strips[0])
    for t in dmas:
        t()
    cur = conv1_alloc()
    for r in range(0, TY + 2, 2):
        conv1_chunk(B1_0, *cur, r)
    B1s = {1: None}
    if ns > 1:
        B1_1, dmas = im2col(*strips[1])
        for t in dmas:
            t()
        B1s[1] = B1_1

    # steady state: conv2(i) interleaved with conv1(i+1) and im2col(i+2).
    # Emission order == tensor-engine execution order, so spreading the
    # conv1 matmuls between conv2 chunks keeps every producer (DVE/scalar/
    # DMA) ahead of its consumer without bursts.
    for i in range(ns):
        n, y0 = strips[i]
        c1work = []
        nxt = None
        if i + 1 < ns:
            nxt = conv1_alloc()
            B1n = B1s.pop(i + 1)
            c1work = [lambda r=r, B1n=B1n, nxt=nxt: conv1_chunk(B1n, *nxt, r)
                      for r in range(0, TY + 2, 2)]
        imwork = []
        if i + 2 < ns:
            B1x, imwork = im2col(*strips[i + 2])
            B1s[i + 2] = B1x
        c2work = [lambda t=t: conv2_chunk(n, y0, *cur, t)
                  for t in range(0, TY, 2)]

        # front-load conv1: two chunks per iteration until exhausted, so the
        # strip's H/H2 writes (DVE/scalar/DMA) finish several iterations
        # before conv2 of the next strip needs them
        ci = 0
        for t in range(len(c2work)):
            for _ in range(2):
                if ci < len(c1work):
                    c1work[ci]()
                    ci += 1
            c2work[t]()
            if t < len(imwork):
                imwork[t]()
        cur = nxt


def build(g: Geom = GEOM, mm_dt=None):
    if mm_dt is None:
        mm_dt = _mm_dt()
    nc = bacc.Bacc("TRN2", target_bir_lowering=False, debug=False,
                   num_devices=N_CORES)
    x = nc.dram_tensor("x", [g.npc, C0, g.h0, g.w0], mm_dt,
                       kind="ExternalInput").ap()
    w1t = nc.dram_tensor("w1t", [27, C1], mm_dt, kind="ExternalInput").ap()
    w2p = nc.dram_tensor("w2p", [128, 3, C2], mm_dt, kind="ExternalInput").ap()
    w2q = nc.dram_tensor("w2q", [128, C2], mm_dt, kind="ExternalInput").ap()
    w2r = nc.dram_tensor("w2r", [C1, C2], mm_dt, kind="ExternalInput").ap()
    out = nc.dram_tensor("out", [g.npc, C2, g.h2, g.w2], mm_dt,
                         kind="ExternalOutput").ap()
    with tile.TileContext(nc) as tc:
        with ExitStack() as ctx:
            _emit(ctx, tc, g, out, x, w1t, w2p, w2q, w2r, mm_dt)
    nc.compile()
    return nc


def host_round(a: np.ndarray) -> np.ndarray:
    """Cast fp32 to the matmul storage dtype (bf16 cast, or tf32 rounding)."""
    a = np.ascontiguousarray(a, dtype=np.float32)
    if MODE == "bf16":
        return a.astype(ml_dtypes.bfloat16)
    b = a.view(np.uint32).copy()
    b += 0xFFF + ((b >> 13) & 1)
    b &= np.uint32(0xFFFFE000)
    return b.view(np.float32)


def pack_weights(w1: np.ndarray, w2: np.ndarray):
    """Host-side repack so every device DMA is contiguous.

    w1t[p, o] = w1[o, c, di, dj] with p = (di*3+dj)*3 + c  (matches im2col)
    w2p[k, dj, o]: k<64 -> w2[o, k, 0, dj];  k>=64 -> w2[o, k-64, 1, dj]
    w2q[k, o]:     k<64 -> w2[o, k, 2, 1];   k>=64 -> w2[o, k-64, 2, 0]
    w2r[c, o] = w2[o, c, 2, 2]
    """
    w1 = np.ascontiguousarray(np.asarray(w1), dtype=np.float32)
    w2 = np.ascontiguousarray(np.asarray(w2), dtype=np.float32)
    w1t = np.ascontiguousarray(w1.transpose(2, 3, 1, 0).reshape(27, C1))
    w2p = np.empty((128, 3, C2), np.float32)
    w2p[:C1] = w2[:, :, 0, :].transpose(1, 2, 0)
    w2p[C1:] = w2[:, :, 1, :].transpose(1, 2, 0)
    w2q = np.empty((128, C2), np.float32)
    w2q[:C1] = w2[:, :, 2, 1].transpose(1, 0)
    w2q[C1:] = w2[:, :, 2, 0].transpose(1, 0)
    w2r = np.ascontiguousarray(w2[:, :, 2, 2].transpose(1, 0))
    return (host_round(w1t), host_round(w2p), host_round(w2q),
            host_round(w2r))


_NC_CACHE: dict = {}


def _get_nc():
    key = ("main", MODE)
    if key not in _NC_CACHE:
        _NC_CACHE[key] = build()
    return _NC_CACHE[key]


def run(x, w1, w2, trace: bool = False):
    """Shard, run on 8 cores, gather.  Returns (out, BassKernelResults)."""
    x = np.ascontiguousarray(np.asarray(x), dtype=np.float32)
    assert x.shape == (FULL_N, C0, GEOM.h0, GEOM.w0), x.shape
    w1t, w2p, w2q, w2r = pack_weights(w1, w2)
    xs = host_round(x).reshape(N_CORES, GEOM.npc, C0, GEOM.h0, GEOM.w0)
    in_maps = [
        {"x": np.ascontiguousarray(xs[c]), "w1t": w1t, "w2p": w2p,
         "w2q": w2q, "w2r": w2r}
        for c in range(N_CORES)
    ]
    nc = _get_nc()
    res = bass_utils.run_bass_kernel_spmd(
        nc, in_maps, core_ids=list(range(N_CORES)), trace=trace)
    out = np.concatenate([r["out"] for r in res.results], axis=0)
    return out.astype(np.float32), res


def kernel(x, w1, w2):
    out, _ = run(x, w1, w2, trace=False)
    return out
